# revision 9
# baseline (speedup 1.0000x reference)
"""Trainium2 Bass kernel for causal multi-head attention (dense transformer block).

Problem: x[2,2048,1024] -> qkv proj -> 16-head causal attention (scale 1/sqrt(1024))
         -> out proj.  8 NeuronCores.

Sharding: core c handles batch b=c//4 and head-group r=c%4 (heads 4r..4r+3).
  - qkv weights column-sharded by head group (q/k/v slices of 256 cols each)
  - attention computed fully on-core in a transposed layout:
      S^T[k,q] = K^T-chunk (stationary) x Q^T (moving) on the PE
      P = exp(S/32) with causal masking; denominator obtained by appending a
      ones-column to V so that O^T = [V|1]^T P gives sums in the last row.
  - AllGather (bf16, groups of 4 cores sharing a batch) assembles all heads'
    outputs feature-major; out-proj is column-sharded with an all-gathered
    feature dim; biases are applied via rank-1 (K=1) matmul accumulation.

Schedule (v2): the serialized SWDGE cast front is broken into 256-token x
casts and per-128-chunk wqkv casts ordered [x0,x1,w0..w7,x2..x7,wo] so the
first qkv matmul starts at ~8us instead of ~24us.  Phase order is
qkv(tb0,tb1) -> attention pass0 -> qkv(tb2,tb3) -> attention pass1, which
hides the tail of the x load under pass-0 compute.  attnV matmuls are
trimmed to the causal column range (no P zero-fill needed).  The per-super
AllGather+out-proj is split into an early AG emission and a later proj
emission; the last pass gathers its first super several jobs before the
stream ends to shorten the serial tail.

kernel(**inputs) takes the FULL fp32 inputs and returns the FULL output.
"""

import sys

sys.path.insert(0, "/opt/trn_rl_repo")

import numpy as np

import concourse.bass as bass
import concourse.bacc as bacc
import concourse.mybir as mybir
import concourse.tile as tile
from concourse.bass import ds, ts
from concourse.bass_utils import run_bass_kernel_spmd
from concourse.masks import make_upper_triangular

F32 = mybir.dt.float32
BF16 = mybir.dt.bfloat16

# ---------------------------------------------------------------- dims
BS, L, DM, H = 2, 2048, 1024, 16
HD = 64                      # head dim
NCORES = 8
GRP = 4                      # cores per batch group (head-parallel)
HLOC = H // GRP              # heads per core = 4
FLOC = HLOC * HD             # local features = 256
SCALE = 1.0 / float(np.sqrt(DM))
REPLICA_GROUPS = [[0, 1, 2, 3], [4, 5, 6, 7]]


class Cfg:
    """Geometry (parametrized so a small config can be tested quickly)."""

    def __init__(self, L=L, DM=DM, hloc=HLOC, hd=HD, npass=2, nwarm=18):
        self.L, self.DM, self.HLOC, self.HD, self.NPASS = L, DM, hloc, hd, npass
        self.FLOC = hloc * hd
        self.NT = L // 128           # 128-token tiles
        self.NB = L // 512           # 512-token blocks
        self.NDM = DM // 128         # dmodel chunks
        self.PW = L // npass         # pass width (q columns per pass)
        self.NSUP = self.PW // 512   # 512-q supers per pass
        self.NFT = self.FLOC // 128  # feature tiles for Q^T/K^T (2)
        self.NWARM = nwarm
        self.scale = 1.0 / float(np.sqrt(DM))
        assert self.PW % 512 == 0 and self.FLOC % 128 == 0


def build_body(nc, cfg, x, wqkv, bq, bk, bv, wo, bo, out, groups):
    """Emit the per-core program (Tile framework) for one iteration."""
    NT, NB, NDM, PW, NSUP, NFT = cfg.NT, cfg.NB, cfg.NDM, cfg.PW, cfg.NSUP, cfg.NFT
    HLOCc, HDc, FLOCc = cfg.HLOC, cfg.HD, cfg.FLOC
    Lc, DMc = cfg.L, cfg.DM
    NPASS = cfg.NPASS
    tc = nc.tc

    with tc.tile_pool(name="const", bufs=1) as constp, \
         tc.tile_pool(name="persist", bufs=1) as pp, \
         tc.tile_pool(name="stage", bufs=3) as sp, \
         tc.tile_pool(name="pbuf", bufs=6) as pbp, \
         tc.tile_pool(name="nrm", bufs=6) as nrm, \
         tc.tile_pool(name="of", bufs=3) as ofp, \
         tc.tile_pool(name="osb", bufs=3) as osbp, \
         tc.tile_pool(name="dram", bufs=2, space="DRAM") as dramp:
        # ---------------- persistent SBUF tensors
        xT = pp.tile([128, NDM, Lc], BF16)                 # x^T  (dm-major)
        wqkvb = pp.tile([128, NDM, 3 * FLOCc], BF16)       # [wq|wk|wv] packed
        wqb = wqkvb[:, :, 0:FLOCc]
        wkb = wqkvb[:, :, FLOCc : 2 * FLOCc]
        wvb = wqkvb[:, :, 2 * FLOCc : 3 * FLOCc]
        wob = pp.tile([128, NDM, FLOCc], BF16)
        QT = pp.tile([128, NFT, Lc], BF16)                 # Q^T feature-major
        KT = pp.tile([128, NFT, Lc], BF16)
        Vb = pp.tile([128, NT, HLOCc * (HDc + 1)], BF16)   # [V | ones] per token tile
        OTs = pp.tile([128, NFT, Lc], BF16)                # attention out^T (feature-major)

        # ---------------- single PSUM pool for the whole kernel
        # bank budget: stile [128,1024]x2 = 4 banks, otile [65,512]x2 = 2,
        # work [128,512]x2 = 2  ->  8 banks.
        psum_cm = tc.tile_pool(name="psum", bufs=2, space="PSUM")
        psum = psum_cm.__enter__()

        # PE warmup: junk matmuls so the p-state ramp happens on the DMA-bound
        # front, not on the first real matmuls.
        NWARM = cfg.NWARM
        wsrc_t = pp.tile([128, 512], BF16, name="wsrc_t")
        nc.vector.memset(wsrc_t, 0.25)
        wps = psum.tile([128, 512], F32, tag="work", name="wps")
        for r in range(NWARM):
            nc.tensor.matmul(wps, wsrc_t[:, 0:128], wsrc_t,
                             start=(r == 0), stop=(r == NWARM - 1))
        wout_t = pp.tile([128, 512], F32, name="wout_t")
        nc.vector.tensor_copy(wout_t, wps[:, 0:512])

        # ---------------- constants (emitted off the Pool DMA path)
        trimask = constp.tile([128, 128], BF16)
        ones_r = constp.tile([1, 128], BF16)
        bq_f = constp.tile([128, NFT], F32)
        bk_f = constp.tile([128, NFT], F32)
        bvb = constp.tile([1, FLOCc], BF16)
        bob = constp.tile([1, FLOCc], BF16)

        def emit_consts():
            make_upper_triangular(nc, trimask, val=1.0, diag=True)
            nc.vector.memset(ones_r, 1.0)
            # biases go over HWDGE (f32) + tiny DVE casts -- keeps the serial
            # Pool SWDGE queue free for the big weight/x cast-loads
            nc.sync.dma_start(bq_f, bq.rearrange("(f p) -> p f", p=128))
            nc.sync.dma_start(bk_f, bk.rearrange("(f p) -> p f", p=128))
            bv_st = constp.tile([1, 2 * FLOCc], F32, name="bv_st")
            nc.sync.dma_start(bv_st[:, 0:FLOCc], bv.rearrange("(a b) -> a b", a=1))
            nc.sync.dma_start(bv_st[:, FLOCc : 2 * FLOCc], bo.rearrange("(a b) -> a b", a=1))
            nc.vector.tensor_copy(bvb, bv_st[:, 0:FLOCc])
            nc.vector.tensor_copy(bob, bv_st[:, FLOCc : 2 * FLOCc])
            # ones columns of Vb
            nc.vector.memset(
                Vb.rearrange("p t (h u) -> p t h u", u=HDc + 1)[:, :, :, HDc : HDc + 1], 1.0
            )

        # ---------------- weight + x staging
        # All casts fp32->bf16 happen inside gpsimd (SWDGE) DMAs on the Pool
        # queue (serial per-DMA desc-gen; transfers serialize on DMA_ENGINES).
        # Order = first-use order: two 256-token x casts feed the transposes
        # gating the first qkv matmuls, then wqkv arrives per 128-row chunk so
        # the c-loop of the first Q/K groups is paced by chunk arrival instead
        # of one monolithic 3MB transfer.
        xv = x.rearrange("(b p2 p) dm -> b p p2 dm", p=128, p2=2)

        def stage_xpair(b2):
            xbf = sp.tile([128, 2, DMc], BF16, tag="xbf", name="xbf")
            nc.gpsimd.dma_start(xbf, xv[b2])
            for k in range(2):
                nc.sync.dma_start(
                    xT[:, :, ts(2 * b2 + k, 128)], xbf[:, k, :], transpose=True
                )

        wv_ = wqkv.rearrange("(c p) f -> c p f", p=128)
        stage_xpair(0)
        stage_xpair(1)
        emit_consts()
        for c in range(NDM):
            nc.gpsimd.dma_start(wqkvb[:, c, :], wv_[c])
        for b2 in range(2, NT // 2):
            stage_xpair(b2)
        nc.gpsimd.dma_start(wob, wo.rearrange("(c p) f -> p c f", p=128))

        # ---------------- qkv projection (one 512-token block)
        def qkv_block(tb):
            qk = [psum.tile([128, 1024], F32, tag="stile", name=f"qk{ft}")
                  for ft in range(NFT)]
            # chunk-major emission: all four Q/K accumulation groups advance
            # together so PE work is available as soon as each w chunk lands
            for c in range(NDM):
                for ft in range(NFT):
                    nc.tensor.matmul(
                        qk[ft][:, 0:512], wqb[:, c, ts(ft, 128)], xT[:, c, ts(tb, 512)],
                        start=(c == 0), stop=(c == NDM - 1),
                    )
                    nc.tensor.matmul(
                        qk[ft][:, 512:1024], wkb[:, c, ts(ft, 128)], xT[:, c, ts(tb, 512)],
                        start=(c == 0), stop=(c == NDM - 1),
                    )
            for ft in range(NFT):
                nc.scalar.activation(QT[:, ft, ts(tb, 512)], qk[ft][:, 0:512],
                                     mybir.ActivationFunctionType.Identity,
                                     bias=bq_f[:, ft : ft + 1])
                nc.scalar.activation(KT[:, ft, ts(tb, 512)], qk[ft][:, 512:1024],
                                     mybir.ActivationFunctionType.Identity,
                                     bias=bk_f[:, ft : ft + 1])
            for tt in range(tb * 4, tb * 4 + 4):
                psv_full = psum.tile([128, 512], F32, tag="work", name="psv_full")
                psv = psv_full[:, 0:FLOCc]
                for c in range(NDM):
                    nc.tensor.matmul(
                        psv, xT[:, c, ts(tt, 128)], wvb[:, c, :],
                        start=(c == 0), stop=False,
                    )
                nc.tensor.matmul(psv, ones_r, bvb, start=False, stop=True)
                # NOTE: Pool/gpsimd has no PSUM access; PSUM reads must go
                # through Act or DVE.
                nc.scalar.copy(
                    Vb[:, tt, :].rearrange("p (h u) -> p h u", u=HDc + 1)[:, :, 0:HDc],
                    psv.rearrange("p (h d) -> p h d", d=HDc),
                )

        # ---------------- attention helpers
        def emit_scores(p, h, i):
            hf, hp = h // 2, h % 2
            S = psum.tile([128, PW], F32, tag="stile", name="S")
            for j2 in range(NSUP):
                qs = p * PW + 512 * j2
                if 128 * i < qs + 512:
                    # causal: columns below the diagonal are never computed
                    al = max(0, 128 * i - qs)
                    nc.tensor.matmul(
                        S[:, ds(512 * j2 + al, 512 - al)],
                        KT[64 * hp : 64 * hp + 64, hf, ts(i, 128)],
                        QT[64 * hp : 64 * hp + 64, hf, ds(qs + al, 512 - al)],
                        start=True, stop=True,
                    )
            return S

        # AllGather + out-proj per 512-token super, split so the collective
        # can be emitted early and the PE-side projection late.
        def emit_ag(p, j2):
            q0 = p * PW + 512 * j2
            ag_in = dramp.tile([NFT * 128, 512], BF16, tag="agin", name="ag_in")
            # NOTE: Shared-output collectives need >4 cores/group; with
            # 4-core groups the output must be a Local scratch tensor.
            ag_out = dramp.tile([GRP * NFT * 128, 512], BF16, tag="agout", name="ag_out")
            for t in range(NFT):
                nc.sync.dma_start(ag_in[ts(t, 128), :], OTs[:, t, ds(q0, 512)])
            nc.gpsimd.collective_compute(
                "AllGather",
                mybir.AluOpType.bypass,
                ins=[ag_in.opt()],
                outs=[ag_out.opt()],
                replica_groups=groups,
            )
            return ag_out

        def emit_proj(p, j2, ag_out):
            q0 = p * PW + 512 * j2
            OF = ofp.tile([128, NDM, 512], BF16, tag="of", name="OF")
            # 2-chunk loads: SP-SEQ DMA issue is ~565ns/DMA, so per-chunk
            # loads would gate the projection on issue rate
            agv = ag_out.rearrange("(c p) q -> p c q", p=128)
            for c2 in range(NDM // 2):
                nc.sync.dma_start(OF[:, 2 * c2 : 2 * c2 + 2, :],
                                  agv[:, 2 * c2 : 2 * c2 + 2, :])
            osb = osbp.tile([128, 4, FLOCc], F32, tag="osb", name="osb")
            outv = out[ds(q0, 512), :].rearrange("(t p) f -> p t f", p=128)
            for ttl in range(4):
                pout_full = psum.tile([128, 512], F32, tag="work", name="pout_full")
                pout = pout_full[:, 0:FLOCc]
                for c in range(NDM):
                    nc.tensor.matmul(
                        pout, OF[:, c, ts(ttl, 128)], wob[:, c, :],
                        start=(c == 0), stop=False,
                    )
                nc.tensor.matmul(pout, ones_r, bob, start=False, stop=True)
                nc.vector.tensor_copy(osb[:, ttl, :], pout)
                if ttl == 1:
                    nc.sync.dma_start(outv[:, 0:2, :], osb[:, 0:2, :])
            nc.sync.dma_start(outv[:, 2:4, :], osb[:, 2:4, :])

        # ---------------- one attention pass (job stream over (head, k-tile))
        def attention_pass(p, inject=None, post=None):
            inject = inject or {}
            ilast = (p + 1) * PW // 128 - 1
            jobs = [(h, i) for h in range(HLOCc) for i in range(ilast + 1)]
            po_all = {}
            # one flat (h, i) stream with scores emitted one step ahead:
            # PE.SEQ is in-order, so S(next) must be issued before attnV(cur)
            # parks the queue on exp(cur) -- including across head boundaries.
            S_next = emit_scores(p, *jobs[0])
            for idx, (h, i) in enumerate(jobs):
                if idx in inject:
                    # fence so the scheduler keeps the AG/proj splice exactly
                    # where the emission order puts it
                    tc.no_sync_barrier()
                for fn in inject.get(idx, ()):
                    fn()
                hf, hp = h // 2, h % 2
                S = S_next
                if idx + 1 < len(jobs):
                    S_next = emit_scores(p, *jobs[idx + 1])
                if i == 0:
                    po_all[h] = [psum.tile([HDc + 1, 512], F32, tag="otile", name="po")
                                 for _ in range(NSUP)]
                po = po_all[h]
                astart = 128 * i - p * PW  # >=0 iff diagonal block in this pass
                es = max(0, astart)
                P = pbp.tile([128, PW], BF16, tag="ptile", name="P")
                nc.scalar.activation(
                    P[:, ds(es, PW - es)],
                    S[:, ds(es, PW - es)],
                    mybir.ActivationFunctionType.Exp,
                    scale=float(cfg.scale),
                )
                if astart >= 0:
                    nc.vector.tensor_mul(P[:, ds(astart, 128)], P[:, ds(astart, 128)], trimask)
                # attnV: trimmed to the causal range [al, 512) per super; the
                # masked sub-diagonal region of P is never read, so no
                # zero-fill of P is needed.  Diagonal super last so the
                # off-diagonal matmuls depend only on exp, not the mask-mul.
                j2s = [j2 for j2 in range(NSUP) if 128 * i < p * PW + 512 * j2 + 512]
                j2s = ([j2 for j2 in j2s if p * PW + 512 * j2 > 128 * i]
                       + [j2 for j2 in j2s if p * PW + 512 * j2 <= 128 * i])
                for j2 in j2s:
                    qs = p * PW + 512 * j2
                    al = max(0, 128 * i - qs)
                    ilastc = min(ilast, (qs + 512) // 128 - 1)
                    nc.tensor.matmul(
                        po[j2][:, ds(al, 512 - al)],
                        Vb[:, i, ds((HDc + 1) * h, HDc + 1)],
                        P[:, ds(512 * j2 + al, 512 - al)],
                        start=(i == 0), stop=(i == ilastc),
                    )
                    if i == ilastc:
                        # the copy exists to free the PSUM accumulator for the
                        # next head; the last head of the last pass normalizes
                        # straight from PSUM (shorter end-of-kernel chain)
                        if p == NPASS - 1 and h == HLOCc - 1:
                            osrc = po[j2]
                        else:
                            osrc = nrm.tile([HDc + 1, 512], F32, tag="osnap", name="osnap")
                            nc.vector.tensor_copy(osrc, po[j2])
                        rec = nrm.tile([1, 512], F32, tag="rec", name="rec")
                        nc.vector.reciprocal(rec, osrc[HDc : HDc + 1, :])
                        rb = nrm.tile([64, 512], F32, tag="rb", name="rb")
                        nc.gpsimd.partition_broadcast(rb, rec)
                        nc.vector.tensor_mul(
                            OTs[64 * hp : 64 * hp + 64, hf, ds(p * PW + 512 * j2, 512)],
                            osrc[0:HDc, :],
                            rb,
                        )
            if post:
                tc.no_sync_barrier()
            for fn in post or ():
                fn()

        # ---------------- phase schedule
        # qkv(tb0,tb1) -> pass0 (q < PW needs only the first PW of tokens)
        # -> qkv(tb2,tb3) (x casts for these blocks land during pass0)
        # -> pass1, with pass0's AG emitted early in the stream, its proj
        # mid-stream, and pass1's own supers drained as soon as each one's
        # last k-tile is done.
        blocks_per_pass = PW // 512
        for tb in range(blocks_per_pass):
            qkv_block(tb)
        aghold = {}

        def mk_ag(p, j2):
            def fn():
                aghold[(p, j2)] = emit_ag(p, j2)
            return fn

        def mk_proj(p, j2):
            def fn():
                emit_proj(p, j2, aghold.pop((p, j2)))
            return fn

        for p in range(NPASS):
            if p > 0:
                # fences: without them the Tile scheduler hoists these qkv
                # blocks before the previous attention pass, parking the
                # in-order PE queue on the late x transposes
                tc.no_sync_barrier()
                for tb in range(p * blocks_per_pass, min((p + 1) * blocks_per_pass, NB)):
                    qkv_block(tb)
                tc.no_sync_barrier()
            ilast = (p + 1) * PW // 128 - 1
            njobs = HLOCc * (ilast + 1)
            inject = {}

            def add(idx, fn):
                idx = max(0, min(idx, njobs - 1))
                inject.setdefault(idx, []).append(fn)

            if p > 0:
                # previous pass's AG right away (collective overlaps compute),
                # projection once the gather has certainly landed
                for j2 in range(NSUP):
                    add(2 + 2 * j2, mk_ag(p - 1, j2))
                add(11 * njobs // 32, mk_proj(p - 1, 0))
                if NSUP > 1:
                    add(21 * njobs // 32, mk_proj(p - 1, 1))
            post = []
            if p == NPASS - 1:
                # drain this pass's own supers: super j2 is complete for all
                # heads once job (h_last, ilastc(j2)) is done
                for j2 in range(NSUP):
                    qs = p * PW + 512 * j2
                    ilc = min(ilast, (qs + 512) // 128 - 1)
                    idx_done = (HLOCc - 1) * (ilast + 1) + ilc + 1
                    if idx_done < njobs:
                        add(idx_done, mk_ag(p, j2))
                        if idx_done + 2 < njobs:
                            add(idx_done + 2, mk_proj(p, j2))
                        else:
                            post.append(mk_proj(p, j2))
                    else:
                        post.append(mk_ag(p, j2))
                        post.append(mk_proj(p, j2))
            attention_pass(p, inject=inject, post=post)
        psum_cm.__exit__(None, None, None)


def make_program(cfg=None, groups=None, unroll=1):
    cfg = cfg or Cfg()
    groups = groups or REPLICA_GROUPS
    nc = bacc.Bacc("TRN2", target_bir_lowering=False, debug=False, num_devices=NCORES)
    x = nc.dram_tensor("x", [cfg.L, cfg.DM], F32, kind="ExternalInput").ap()
    wqkv = nc.dram_tensor("wqkv", [cfg.DM, 3 * cfg.FLOC], F32, kind="ExternalInput").ap()
    bq = nc.dram_tensor("bq", [cfg.FLOC], F32, kind="ExternalInput").ap()
    bk = nc.dram_tensor("bk", [cfg.FLOC], F32, kind="ExternalInput").ap()
    bv = nc.dram_tensor("bv", [cfg.FLOC], F32, kind="ExternalInput").ap()
    wo = nc.dram_tensor("wo", [cfg.DM, cfg.FLOC], F32, kind="ExternalInput").ap()
    bo = nc.dram_tensor("bo", [cfg.FLOC], F32, kind="ExternalInput").ap()
    out = nc.dram_tensor("out", [cfg.L, cfg.FLOC], F32, kind="ExternalOutput").ap()
    with tile.TileContext(nc) as tc:
        nc.tc = tc
        for _ in range(unroll):
            build_body(nc, cfg, x, wqkv, bq, bk, bv, wo, bo, out, groups)
    nc.compile()
    return nc


def shard_inputs(x, w_qkv, b_qkv, w_out, b_out, cfg=None):
    """Full inputs -> list of 8 per-core input dicts."""
    cfg = cfg or Cfg()
    FL = cfg.FLOC
    DMF = cfg.DM
    in_maps = []
    for c in range(NCORES):
        b, r = divmod(c, GRP)
        q0 = r * FL
        in_maps.append({
            "x": np.ascontiguousarray(x[b]),
            "wqkv": np.ascontiguousarray(np.concatenate([
                w_qkv[:, q0 : q0 + FL],
                w_qkv[:, DMF + q0 : DMF + q0 + FL],
                w_qkv[:, 2 * DMF + q0 : 2 * DMF + q0 + FL],
            ], axis=1)),
            "bq": np.ascontiguousarray(b_qkv[q0 : q0 + FL]),
            "bk": np.ascontiguousarray(b_qkv[DMF + q0 : DMF + q0 + FL]),
            "bv": np.ascontiguousarray(b_qkv[2 * DMF + q0 : 2 * DMF + q0 + FL]),
            "wo": np.ascontiguousarray(w_out[:, q0 : q0 + FL]),
            "bo": np.ascontiguousarray(b_out[q0 : q0 + FL]),
        })
    return in_maps


def gather_output(results, cfg=None):
    cfg = cfg or Cfg()
    FL = cfg.FLOC
    out = np.empty((BS, cfg.L, cfg.DM), np.float32)
    for c in range(NCORES):
        b, r = divmod(c, GRP)
        out[b, :, r * FL : (r + 1) * FL] = results[c]["out"]
    return out


_PROGRAM = None


def _get_program():
    global _PROGRAM
    if _PROGRAM is None:
        _PROGRAM = make_program()
    return _PROGRAM


def kernel(x, w_qkv, b_qkv, w_out, b_out):
    x = np.asarray(x, np.float32)
    w_qkv = np.asarray(w_qkv, np.float32)
    b_qkv = np.asarray(b_qkv, np.float32)
    w_out = np.asarray(w_out, np.float32)
    b_out = np.asarray(b_out, np.float32)
    nc = _get_program()
    in_maps = shard_inputs(x, w_qkv, b_qkv, w_out, b_out)
    res = run_bass_kernel_spmd(nc, in_maps, list(range(NCORES)))
    return gather_output(res.results)


# revision 12
# speedup vs baseline: 1.0106x; 1.0106x over previous
"""Trainium2 Bass kernel for causal multi-head attention (dense transformer block).

Problem: x[2,2048,1024] -> qkv proj -> 16-head causal attention (scale 1/sqrt(1024))
         -> out proj.  8 NeuronCores.

Sharding: core c handles batch b=c//4 and head-group r=c%4 (heads 4r..4r+3).
  - qkv weights column-sharded by head group (q/k/v slices of 256 cols each)
  - attention computed fully on-core in a transposed layout:
      S^T[k,q] = K^T-chunk (stationary) x Q^T (moving) on the PE
      P = exp(S/32) with causal masking; denominator obtained by appending a
      ones-column to V so that O^T = [V|1]^T P gives sums in the last row.
  - AllGather (bf16, groups of 4 cores sharing a batch) assembles all heads'
    outputs feature-major; out-proj is column-sharded with an all-gathered
    feature dim; biases are applied via rank-1 (K=1) matmul accumulation.

Schedule (v2): the serialized SWDGE cast front is broken into 256-token x
casts and per-128-chunk wqkv casts ordered [x0,x1,w0..w7,x2..x7,wo] so the
first qkv matmul starts at ~8us instead of ~24us.  Phase order is
qkv(tb0,tb1) -> attention pass0 -> qkv(tb2,tb3) -> attention pass1, which
hides the tail of the x load under pass-0 compute.  attnV matmuls are
trimmed to the causal column range (no P zero-fill needed).  The per-super
AllGather+out-proj is split into an early AG emission and a later proj
emission; the last pass gathers its first super several jobs before the
stream ends to shorten the serial tail.

kernel(**inputs) takes the FULL fp32 inputs and returns the FULL output.
"""

import sys

sys.path.insert(0, "/opt/trn_rl_repo")

import numpy as np

import concourse.bass as bass
import concourse.bacc as bacc
import concourse.mybir as mybir
import concourse.tile as tile
from concourse.bass import ds, ts
from concourse.bass_utils import run_bass_kernel_spmd
from concourse.masks import make_upper_triangular

F32 = mybir.dt.float32
BF16 = mybir.dt.bfloat16

# ---------------------------------------------------------------- dims
BS, L, DM, H = 2, 2048, 1024, 16
HD = 64                      # head dim
NCORES = 8
GRP = 4                      # cores per batch group (head-parallel)
HLOC = H // GRP              # heads per core = 4
FLOC = HLOC * HD             # local features = 256
SCALE = 1.0 / float(np.sqrt(DM))
REPLICA_GROUPS = [[0, 1, 2, 3], [4, 5, 6, 7]]


class Cfg:
    """Geometry (parametrized so a small config can be tested quickly)."""

    def __init__(self, L=L, DM=DM, hloc=HLOC, hd=HD, npass=2, nwarm=18):
        self.L, self.DM, self.HLOC, self.HD, self.NPASS = L, DM, hloc, hd, npass
        self.FLOC = hloc * hd
        self.NT = L // 128           # 128-token tiles
        self.NB = L // 512           # 512-token blocks
        self.NDM = DM // 128         # dmodel chunks
        self.PW = L // npass         # pass width (q columns per pass)
        self.NSUP = self.PW // 512   # 512-q supers per pass
        self.NFT = self.FLOC // 128  # feature tiles for Q^T/K^T (2)
        self.NWARM = nwarm
        self.scale = 1.0 / float(np.sqrt(DM))
        assert self.PW % 512 == 0 and self.FLOC % 128 == 0


def build_body(nc, cfg, x, wqkv, bq, bk, bv, wo, bo, out, groups):
    """Emit the per-core program (Tile framework) for one iteration."""
    NT, NB, NDM, PW, NSUP, NFT = cfg.NT, cfg.NB, cfg.NDM, cfg.PW, cfg.NSUP, cfg.NFT
    HLOCc, HDc, FLOCc = cfg.HLOC, cfg.HD, cfg.FLOC
    Lc, DMc = cfg.L, cfg.DM
    NPASS = cfg.NPASS
    tc = nc.tc

    with tc.tile_pool(name="const", bufs=1) as constp, \
         tc.tile_pool(name="persist", bufs=1) as pp, \
         tc.tile_pool(name="stage", bufs=3) as sp, \
         tc.tile_pool(name="pbuf", bufs=6) as pbp, \
         tc.tile_pool(name="nrm", bufs=6) as nrm, \
         tc.tile_pool(name="of", bufs=3) as ofp, \
         tc.tile_pool(name="osb", bufs=3) as osbp, \
         tc.tile_pool(name="dram", bufs=2, space="DRAM") as dramp:
        # ---------------- persistent SBUF tensors
        xT = pp.tile([128, NDM, Lc], BF16)                 # x^T  (dm-major)
        wqkvb = pp.tile([128, NDM, 3 * FLOCc], BF16)       # [wq|wk|wv] packed
        wqb = wqkvb[:, :, 0:FLOCc]
        wkb = wqkvb[:, :, FLOCc : 2 * FLOCc]
        wvb = wqkvb[:, :, 2 * FLOCc : 3 * FLOCc]
        wob = pp.tile([128, NDM, FLOCc], BF16)
        QT = pp.tile([128, NFT, Lc], BF16)                 # Q^T feature-major
        KT = pp.tile([128, NFT, Lc], BF16)
        Vb = pp.tile([128, NT, HLOCc * (HDc + 1)], BF16)   # [V | ones] per token tile
        OTs = pp.tile([128, NFT, Lc], BF16)                # attention out^T (feature-major)

        # ---------------- single PSUM pool for the whole kernel
        # bank budget: stile [128,1024]x2 = 4 banks, otile [65,512]x2 = 2,
        # work [128,512]x2 = 2  ->  8 banks.
        psum_cm = tc.tile_pool(name="psum", bufs=2, space="PSUM")
        psum = psum_cm.__enter__()

        # PE warmup: junk matmuls so the p-state ramp happens on the DMA-bound
        # front, not on the first real matmuls.
        NWARM = cfg.NWARM
        wsrc_t = pp.tile([128, 512], BF16, name="wsrc_t")
        nc.vector.memset(wsrc_t, 0.25)
        wps = psum.tile([128, 512], F32, tag="work", name="wps")
        for r in range(NWARM):
            nc.tensor.matmul(wps, wsrc_t[:, 0:128], wsrc_t,
                             start=(r == 0), stop=(r == NWARM - 1))
        wout_t = pp.tile([128, 512], F32, name="wout_t")
        nc.vector.tensor_copy(wout_t, wps[:, 0:512])

        # ---------------- constants (emitted off the Pool DMA path)
        trimask = constp.tile([128, 128], BF16)
        ones_r = constp.tile([1, 128], BF16)
        bq_f = constp.tile([128, NFT], F32)
        bk_f = constp.tile([128, NFT], F32)
        bvb = constp.tile([1, FLOCc], BF16)
        bob = constp.tile([1, FLOCc], BF16)

        def emit_consts():
            make_upper_triangular(nc, trimask, val=1.0, diag=True)
            nc.vector.memset(ones_r, 1.0)
            # biases go over the Act HWDGE path (f32) + tiny DVE casts --
            # keeps them off the Pool SWDGE cast queue AND off the SP queue
            # whose HW DMA queues fill with x transposes (a const queued
            # behind 12 transposes stalls every Act bias-copy transitively)
            nc.scalar.dma_start(bq_f, bq.rearrange("(f p) -> p f", p=128))
            nc.scalar.dma_start(bk_f, bk.rearrange("(f p) -> p f", p=128))
            bv_st = constp.tile([1, 2 * FLOCc], F32, name="bv_st")
            nc.scalar.dma_start(bv_st[:, 0:FLOCc], bv.rearrange("(a b) -> a b", a=1))
            nc.scalar.dma_start(bv_st[:, FLOCc : 2 * FLOCc], bo.rearrange("(a b) -> a b", a=1))
            nc.vector.tensor_copy(bvb, bv_st[:, 0:FLOCc])
            nc.vector.tensor_copy(bob, bv_st[:, FLOCc : 2 * FLOCc])
            # ones columns of Vb
            nc.vector.memset(
                Vb.rearrange("p t (h u) -> p t h u", u=HDc + 1)[:, :, :, HDc : HDc + 1], 1.0
            )

        # ---------------- weight + x staging
        # All casts fp32->bf16 happen inside gpsimd (SWDGE) DMAs on the Pool
        # queue (serial per-DMA desc-gen; transfers serialize on DMA_ENGINES).
        # Order = first-use order: two 256-token x casts feed the transposes
        # gating the first qkv matmuls, then wqkv arrives per 128-row chunk so
        # the c-loop of the first Q/K groups is paced by chunk arrival instead
        # of one monolithic 3MB transfer.
        xv = x.rearrange("(b p2 p) dm -> b p p2 dm", p=128, p2=2)

        def stage_xpair(b2):
            xbf = sp.tile([128, 2, DMc], BF16, tag="xbf", name="xbf")
            nc.gpsimd.dma_start(xbf, xv[b2])
            for k in range(2):
                nc.sync.dma_start(
                    xT[:, :, ts(2 * b2 + k, 128)], xbf[:, k, :], transpose=True
                )

        wv_ = wqkv.rearrange("(c p) f -> c p f", p=128)
        emit_consts()
        stage_xpair(0)
        stage_xpair(1)
        for c in range(NDM):
            nc.gpsimd.dma_start(wqkvb[:, c, :], wv_[c])
        for b2 in range(2, NT // 2):
            stage_xpair(b2)
        nc.gpsimd.dma_start(wob, wo.rearrange("(c p) f -> p c f", p=128))

        # ---------------- qkv projection (one 512-token block)
        def qkv_block(tb):
            qk = [psum.tile([128, 1024], F32, tag="stile", name=f"qk{ft}")
                  for ft in range(NFT)]
            # chunk-major emission: all four Q/K accumulation groups advance
            # together so PE work is available as soon as each w chunk lands
            for c in range(NDM):
                for ft in range(NFT):
                    nc.tensor.matmul(
                        qk[ft][:, 0:512], wqb[:, c, ts(ft, 128)], xT[:, c, ts(tb, 512)],
                        start=(c == 0), stop=(c == NDM - 1),
                    )
                    nc.tensor.matmul(
                        qk[ft][:, 512:1024], wkb[:, c, ts(ft, 128)], xT[:, c, ts(tb, 512)],
                        start=(c == 0), stop=(c == NDM - 1),
                    )
            for ft in range(NFT):
                nc.scalar.activation(QT[:, ft, ts(tb, 512)], qk[ft][:, 0:512],
                                     mybir.ActivationFunctionType.Identity,
                                     bias=bq_f[:, ft : ft + 1])
                nc.scalar.activation(KT[:, ft, ts(tb, 512)], qk[ft][:, 512:1024],
                                     mybir.ActivationFunctionType.Identity,
                                     bias=bk_f[:, ft : ft + 1])
            for tt in range(tb * 4, tb * 4 + 4):
                psv_full = psum.tile([128, 512], F32, tag="work", name="psv_full")
                psv = psv_full[:, 0:FLOCc]
                for c in range(NDM):
                    nc.tensor.matmul(
                        psv, xT[:, c, ts(tt, 128)], wvb[:, c, :],
                        start=(c == 0), stop=False,
                    )
                nc.tensor.matmul(psv, ones_r, bvb, start=False, stop=True)
                # NOTE: Pool/gpsimd has no PSUM access; PSUM reads must go
                # through Act or DVE.
                nc.scalar.copy(
                    Vb[:, tt, :].rearrange("p (h u) -> p h u", u=HDc + 1)[:, :, 0:HDc],
                    psv.rearrange("p (h d) -> p h d", d=HDc),
                )

        # ---------------- attention helpers
        def emit_scores(p, h, i):
            hf, hp = h // 2, h % 2
            S = psum.tile([128, PW], F32, tag="stile", name="S")
            for j2 in range(NSUP):
                qs = p * PW + 512 * j2
                if 128 * i < qs + 512:
                    # causal: columns below the diagonal are never computed
                    al = max(0, 128 * i - qs)
                    nc.tensor.matmul(
                        S[:, ds(512 * j2 + al, 512 - al)],
                        KT[64 * hp : 64 * hp + 64, hf, ts(i, 128)],
                        QT[64 * hp : 64 * hp + 64, hf, ds(qs + al, 512 - al)],
                        start=True, stop=True,
                    )
            return S

        # AllGather + out-proj per 512-token super, split so the collective
        # can be emitted early and the PE-side projection late.
        def emit_ag(p, j2):
            q0 = p * PW + 512 * j2
            ag_in = dramp.tile([NFT * 128, 512], BF16, tag="agin", name="ag_in")
            # NOTE: Shared-output collectives need >4 cores/group; with
            # 4-core groups the output must be a Local scratch tensor.
            ag_out = dramp.tile([GRP * NFT * 128, 512], BF16, tag="agout", name="ag_out")
            for t in range(NFT):
                nc.sync.dma_start(ag_in[ts(t, 128), :], OTs[:, t, ds(q0, 512)])
            nc.gpsimd.collective_compute(
                "AllGather",
                mybir.AluOpType.bypass,
                ins=[ag_in.opt()],
                outs=[ag_out.opt()],
                replica_groups=groups,
            )
            return ag_out

        def emit_proj(p, j2, ag_out):
            q0 = p * PW + 512 * j2
            OF = ofp.tile([128, NDM, 512], BF16, tag="of", name="OF")
            # 2-chunk loads: SP-SEQ DMA issue is ~565ns/DMA, so per-chunk
            # loads would gate the projection on issue rate
            agv = ag_out.rearrange("(c p) q -> p c q", p=128)
            for c2 in range(NDM // 2):
                nc.sync.dma_start(OF[:, 2 * c2 : 2 * c2 + 2, :],
                                  agv[:, 2 * c2 : 2 * c2 + 2, :])
            osb = osbp.tile([128, 4, FLOCc], F32, tag="osb", name="osb")
            outv = out[ds(q0, 512), :].rearrange("(t p) f -> p t f", p=128)
            for ttl in range(4):
                pout_full = psum.tile([128, 512], F32, tag="work", name="pout_full")
                pout = pout_full[:, 0:FLOCc]
                for c in range(NDM):
                    nc.tensor.matmul(
                        pout, OF[:, c, ts(ttl, 128)], wob[:, c, :],
                        start=(c == 0), stop=False,
                    )
                nc.tensor.matmul(pout, ones_r, bob, start=False, stop=True)
                nc.vector.tensor_copy(osb[:, ttl, :], pout)
                if ttl == 1:
                    nc.sync.dma_start(outv[:, 0:2, :], osb[:, 0:2, :])
            nc.sync.dma_start(outv[:, 2:4, :], osb[:, 2:4, :])

        # ---------------- one attention pass (job stream over (head, k-tile))
        def attention_pass(p, inject=None, post=None):
            inject = inject or {}
            ilast = (p + 1) * PW // 128 - 1
            jobs = [(h, i) for h in range(HLOCc) for i in range(ilast + 1)]
            po_all = {}
            # one flat (h, i) stream with scores emitted one step ahead:
            # PE.SEQ is in-order, so S(next) must be issued before attnV(cur)
            # parks the queue on exp(cur) -- including across head boundaries.
            S_next = emit_scores(p, *jobs[0])
            for idx, (h, i) in enumerate(jobs):
                if idx in inject:
                    # fence so the scheduler keeps the AG/proj splice exactly
                    # where the emission order puts it
                    tc.no_sync_barrier()
                for fn in inject.get(idx, ()):
                    fn()
                hf, hp = h // 2, h % 2
                S = S_next
                if idx + 1 < len(jobs):
                    S_next = emit_scores(p, *jobs[idx + 1])
                if i == 0:
                    po_all[h] = [psum.tile([HDc + 1, 512], F32, tag="otile", name="po")
                                 for _ in range(NSUP)]
                po = po_all[h]
                astart = 128 * i - p * PW  # >=0 iff diagonal block in this pass
                es = max(0, astart)
                P = pbp.tile([128, PW], BF16, tag="ptile", name="P")
                nc.scalar.activation(
                    P[:, ds(es, PW - es)],
                    S[:, ds(es, PW - es)],
                    mybir.ActivationFunctionType.Exp,
                    scale=float(cfg.scale),
                )
                if astart >= 0:
                    nc.vector.tensor_mul(P[:, ds(astart, 128)], P[:, ds(astart, 128)], trimask)
                # attnV: trimmed to the causal range [al, 512) per super; the
                # masked sub-diagonal region of P is never read, so no
                # zero-fill of P is needed.  Diagonal super last so the
                # off-diagonal matmuls depend only on exp, not the mask-mul.
                j2s = [j2 for j2 in range(NSUP) if 128 * i < p * PW + 512 * j2 + 512]
                j2s = ([j2 for j2 in j2s if p * PW + 512 * j2 > 128 * i]
                       + [j2 for j2 in j2s if p * PW + 512 * j2 <= 128 * i])
                for j2 in j2s:
                    qs = p * PW + 512 * j2
                    al = max(0, 128 * i - qs)
                    ilastc = min(ilast, (qs + 512) // 128 - 1)
                    nc.tensor.matmul(
                        po[j2][:, ds(al, 512 - al)],
                        Vb[:, i, ds((HDc + 1) * h, HDc + 1)],
                        P[:, ds(512 * j2 + al, 512 - al)],
                        start=(i == 0), stop=(i == ilastc),
                    )
                    if i == ilastc:
                        # the copy exists to free the PSUM accumulator for the
                        # next head; the last head of the last pass normalizes
                        # straight from PSUM (shorter end-of-kernel chain)
                        if p == NPASS - 1 and h == HLOCc - 1:
                            osrc = po[j2]
                        else:
                            osrc = nrm.tile([HDc + 1, 512], F32, tag="osnap", name="osnap")
                            nc.vector.tensor_copy(osrc, po[j2])
                        rec = nrm.tile([1, 512], F32, tag="rec", name="rec")
                        nc.vector.reciprocal(rec, osrc[HDc : HDc + 1, :])
                        rb = nrm.tile([64, 512], F32, tag="rb", name="rb")
                        nc.gpsimd.partition_broadcast(rb, rec)
                        nc.vector.tensor_mul(
                            OTs[64 * hp : 64 * hp + 64, hf, ds(p * PW + 512 * j2, 512)],
                            osrc[0:HDc, :],
                            rb,
                        )
            if post:
                tc.no_sync_barrier()
            for fn in post or ():
                fn()

        # ---------------- phase schedule
        # qkv(tb0,tb1) -> pass0 (q < PW needs only the first PW of tokens)
        # -> qkv(tb2,tb3) (x casts for these blocks land during pass0)
        # -> pass1, with pass0's AG emitted early in the stream, its proj
        # mid-stream, and pass1's own supers drained as soon as each one's
        # last k-tile is done.
        blocks_per_pass = PW // 512
        for tb in range(blocks_per_pass):
            qkv_block(tb)
        aghold = {}

        def mk_ag(p, j2):
            def fn():
                aghold[(p, j2)] = emit_ag(p, j2)
            return fn

        def mk_proj(p, j2):
            def fn():
                emit_proj(p, j2, aghold.pop((p, j2)))
            return fn

        for p in range(NPASS):
            if p > 0:
                # fences: without them the Tile scheduler hoists these qkv
                # blocks before the previous attention pass, parking the
                # in-order PE queue on the late x transposes
                tc.no_sync_barrier()
                for tb in range(p * blocks_per_pass, min((p + 1) * blocks_per_pass, NB)):
                    qkv_block(tb)
                tc.no_sync_barrier()
            ilast = (p + 1) * PW // 128 - 1
            njobs = HLOCc * (ilast + 1)
            inject = {}

            def add(idx, fn):
                idx = max(0, min(idx, njobs - 1))
                inject.setdefault(idx, []).append(fn)

            if p > 0:
                # previous pass's AG right away (collective overlaps compute),
                # projection once the gather has certainly landed
                for j2 in range(NSUP):
                    add(2 + 2 * j2, mk_ag(p - 1, j2))
                add(11 * njobs // 32, mk_proj(p - 1, 0))
                if NSUP > 1:
                    add(21 * njobs // 32, mk_proj(p - 1, 1))
            post = []
            if p == NPASS - 1:
                # drain this pass's own supers: super j2 is complete for all
                # heads once job (h_last, ilastc(j2)) is done
                for j2 in range(NSUP):
                    qs = p * PW + 512 * j2
                    ilc = min(ilast, (qs + 512) // 128 - 1)
                    idx_done = (HLOCc - 1) * (ilast + 1) + ilc + 1
                    if idx_done < njobs:
                        add(idx_done, mk_ag(p, j2))
                        if idx_done + 2 < njobs:
                            add(idx_done + 2, mk_proj(p, j2))
                        else:
                            post.append(mk_proj(p, j2))
                    else:
                        post.append(mk_ag(p, j2))
                        post.append(mk_proj(p, j2))
            attention_pass(p, inject=inject, post=post)
        psum_cm.__exit__(None, None, None)


def make_program(cfg=None, groups=None, unroll=1):
    cfg = cfg or Cfg()
    groups = groups or REPLICA_GROUPS
    nc = bacc.Bacc("TRN2", target_bir_lowering=False, debug=False, num_devices=NCORES)
    x = nc.dram_tensor("x", [cfg.L, cfg.DM], F32, kind="ExternalInput").ap()
    wqkv = nc.dram_tensor("wqkv", [cfg.DM, 3 * cfg.FLOC], F32, kind="ExternalInput").ap()
    bq = nc.dram_tensor("bq", [cfg.FLOC], F32, kind="ExternalInput").ap()
    bk = nc.dram_tensor("bk", [cfg.FLOC], F32, kind="ExternalInput").ap()
    bv = nc.dram_tensor("bv", [cfg.FLOC], F32, kind="ExternalInput").ap()
    wo = nc.dram_tensor("wo", [cfg.DM, cfg.FLOC], F32, kind="ExternalInput").ap()
    bo = nc.dram_tensor("bo", [cfg.FLOC], F32, kind="ExternalInput").ap()
    out = nc.dram_tensor("out", [cfg.L, cfg.FLOC], F32, kind="ExternalOutput").ap()
    with tile.TileContext(nc) as tc:
        nc.tc = tc
        for _ in range(unroll):
            build_body(nc, cfg, x, wqkv, bq, bk, bv, wo, bo, out, groups)
    nc.compile()
    return nc


def shard_inputs(x, w_qkv, b_qkv, w_out, b_out, cfg=None):
    """Full inputs -> list of 8 per-core input dicts."""
    cfg = cfg or Cfg()
    FL = cfg.FLOC
    DMF = cfg.DM
    in_maps = []
    for c in range(NCORES):
        b, r = divmod(c, GRP)
        q0 = r * FL
        in_maps.append({
            "x": np.ascontiguousarray(x[b]),
            "wqkv": np.ascontiguousarray(np.concatenate([
                w_qkv[:, q0 : q0 + FL],
                w_qkv[:, DMF + q0 : DMF + q0 + FL],
                w_qkv[:, 2 * DMF + q0 : 2 * DMF + q0 + FL],
            ], axis=1)),
            "bq": np.ascontiguousarray(b_qkv[q0 : q0 + FL]),
            "bk": np.ascontiguousarray(b_qkv[DMF + q0 : DMF + q0 + FL]),
            "bv": np.ascontiguousarray(b_qkv[2 * DMF + q0 : 2 * DMF + q0 + FL]),
            "wo": np.ascontiguousarray(w_out[:, q0 : q0 + FL]),
            "bo": np.ascontiguousarray(b_out[q0 : q0 + FL]),
        })
    return in_maps


def gather_output(results, cfg=None):
    cfg = cfg or Cfg()
    FL = cfg.FLOC
    out = np.empty((BS, cfg.L, cfg.DM), np.float32)
    for c in range(NCORES):
        b, r = divmod(c, GRP)
        out[b, :, r * FL : (r + 1) * FL] = results[c]["out"]
    return out


_PROGRAM = None


def _get_program():
    global _PROGRAM
    if _PROGRAM is None:
        _PROGRAM = make_program()
    return _PROGRAM


def kernel(x, w_qkv, b_qkv, w_out, b_out):
    x = np.asarray(x, np.float32)
    w_qkv = np.asarray(w_qkv, np.float32)
    b_qkv = np.asarray(b_qkv, np.float32)
    w_out = np.asarray(w_out, np.float32)
    b_out = np.asarray(b_out, np.float32)
    nc = _get_program()
    in_maps = shard_inputs(x, w_qkv, b_qkv, w_out, b_out)
    res = run_bass_kernel_spmd(nc, in_maps, list(range(NCORES)))
    return gather_output(res.results)


# revision 16
# speedup vs baseline: 1.0548x; 1.0437x over previous
"""Trainium2 Bass kernel for causal multi-head attention (dense transformer block).

Problem: x[2,2048,1024] -> qkv proj -> 16-head causal attention (scale 1/sqrt(1024))
         -> out proj.  8 NeuronCores.

Sharding: core c handles batch b=c//4 and head-group r=c%4 (heads 4r..4r+3).
  - qkv weights column-sharded by head group (q/k/v slices of 256 cols each)
  - attention computed fully on-core in a transposed layout:
      S^T[k,q] = K^T-chunk (stationary) x Q^T (moving) on the PE
      P = exp(S/32) with causal masking; denominator obtained by appending a
      ones-column to V so that O^T = [V|1]^T P gives sums in the last row.
  - AllGather (bf16, groups of 4 cores sharing a batch) assembles all heads'
    outputs feature-major; out-proj is column-sharded with an all-gathered
    feature dim; biases are applied via rank-1 (K=1) matmul accumulation.

Schedule (v2): the serialized SWDGE cast front is broken into 256-token x
casts and per-128-chunk wqkv casts ordered [x0,x1,w0..w7,x2..x7,wo] so the
first qkv matmul starts at ~8us instead of ~24us.  Phase order is
qkv(tb0,tb1) -> attention pass0 -> qkv(tb2,tb3) -> attention pass1, which
hides the tail of the x load under pass-0 compute.  attnV matmuls are
trimmed to the causal column range (no P zero-fill needed).  The per-super
AllGather+out-proj is split into an early AG emission and a later proj
emission; the last pass gathers its first super several jobs before the
stream ends to shorten the serial tail.

kernel(**inputs) takes the FULL fp32 inputs and returns the FULL output.
"""

import sys

sys.path.insert(0, "/opt/trn_rl_repo")

import numpy as np

import concourse.bass as bass
import concourse.bacc as bacc
import concourse.mybir as mybir
import concourse.tile as tile
from concourse.bass import ds, ts
from concourse.bass_utils import run_bass_kernel_spmd
from concourse.masks import make_upper_triangular

F32 = mybir.dt.float32
BF16 = mybir.dt.bfloat16

# ---------------------------------------------------------------- dims
BS, L, DM, H = 2, 2048, 1024, 16
HD = 64                      # head dim
NCORES = 8
GRP = 4                      # cores per batch group (head-parallel)
HLOC = H // GRP              # heads per core = 4
FLOC = HLOC * HD             # local features = 256
SCALE = 1.0 / float(np.sqrt(DM))
REPLICA_GROUPS = [[0, 1, 2, 3], [4, 5, 6, 7]]


class Cfg:
    """Geometry (parametrized so a small config can be tested quickly)."""

    def __init__(self, L=L, DM=DM, hloc=HLOC, hd=HD, npass=2, nwarm=18):
        self.L, self.DM, self.HLOC, self.HD, self.NPASS = L, DM, hloc, hd, npass
        self.FLOC = hloc * hd
        self.NT = L // 128           # 128-token tiles
        self.NB = L // 512           # 512-token blocks
        self.NDM = DM // 128         # dmodel chunks
        self.PW = L // npass         # pass width (q columns per pass)
        self.NSUP = self.PW // 512   # 512-q supers per pass
        self.NFT = self.FLOC // 128  # feature tiles for Q^T/K^T (2)
        self.NWARM = nwarm
        self.scale = 1.0 / float(np.sqrt(DM))
        assert self.PW % 512 == 0 and self.FLOC % 128 == 0


def build_body(nc, cfg, x, wqkv, bq, bk, bv, wo, bo, out, groups):
    """Emit the per-core program (Tile framework) for one iteration."""
    NT, NB, NDM, PW, NSUP, NFT = cfg.NT, cfg.NB, cfg.NDM, cfg.PW, cfg.NSUP, cfg.NFT
    HLOCc, HDc, FLOCc = cfg.HLOC, cfg.HD, cfg.FLOC
    Lc, DMc = cfg.L, cfg.DM
    NPASS = cfg.NPASS
    tc = nc.tc

    with tc.tile_pool(name="const", bufs=1) as constp, \
         tc.tile_pool(name="persist", bufs=1) as pp, \
         tc.tile_pool(name="stage", bufs=3) as sp, \
         tc.tile_pool(name="pbuf", bufs=6) as pbp, \
         tc.tile_pool(name="nrm", bufs=6) as nrm, \
         tc.tile_pool(name="of", bufs=3) as ofp, \
         tc.tile_pool(name="osb", bufs=3) as osbp, \
         tc.tile_pool(name="dram", bufs=2, space="DRAM") as dramp:
        # ---------------- persistent SBUF tensors
        xT = pp.tile([128, NDM, Lc], BF16)                 # x^T  (dm-major)
        wqkvb = pp.tile([128, NDM, 3 * FLOCc], BF16)       # [wq|wk|wv] packed
        wqb = wqkvb[:, :, 0:FLOCc]
        wkb = wqkvb[:, :, FLOCc : 2 * FLOCc]
        wvb = wqkvb[:, :, 2 * FLOCc : 3 * FLOCc]
        wob = pp.tile([128, NDM, FLOCc], BF16)
        QT = pp.tile([128, NFT, Lc], BF16)                 # Q^T feature-major
        KT = pp.tile([128, NFT, Lc], BF16)
        Vb = pp.tile([128, NT, HLOCc * (HDc + 1)], BF16)   # [V | ones] per token tile
        OTs = pp.tile([128, NFT, Lc], BF16)                # attention out^T (feature-major)

        # ---------------- single PSUM pool for the whole kernel
        # bank budget: stile [128,1024]x2 = 4 banks, otile [65,512]x2 = 2,
        # work [128,512]x2 = 2  ->  8 banks.
        psum_cm = tc.tile_pool(name="psum", bufs=2, space="PSUM")
        psum = psum_cm.__enter__()

        # PE warmup: junk matmuls so the p-state ramp happens on the DMA-bound
        # front, not on the first real matmuls.
        NWARM = cfg.NWARM
        wsrc_t = pp.tile([128, 512], BF16, name="wsrc_t")
        nc.vector.memset(wsrc_t, 0.25)
        wps = psum.tile([128, 512], F32, tag="work", name="wps")
        for r in range(NWARM):
            nc.tensor.matmul(wps, wsrc_t[:, 0:128], wsrc_t,
                             start=(r == 0), stop=(r == NWARM - 1))
        wout_t = pp.tile([128, 512], F32, name="wout_t")
        nc.vector.tensor_copy(wout_t, wps[:, 0:512])

        # ---------------- constants (emitted off the Pool DMA path)
        trimask = constp.tile([128, 128], BF16)
        ones_r = constp.tile([1, 128], BF16)
        bq_f = constp.tile([128, NFT], F32)
        bk_f = constp.tile([128, NFT], F32)
        bvb = constp.tile([1, FLOCc], BF16)
        bob = constp.tile([1, FLOCc], BF16)

        def emit_consts():
            make_upper_triangular(nc, trimask, val=1.0, diag=True)
            nc.vector.memset(ones_r, 1.0)
            # biases go over the Act HWDGE path (f32) + tiny DVE casts --
            # keeps them off the Pool SWDGE cast queue AND off the SP queue
            # whose HW DMA queues fill with x transposes (a const queued
            # behind 12 transposes stalls every Act bias-copy transitively)
            nc.scalar.dma_start(bq_f, bq.rearrange("(f p) -> p f", p=128))
            nc.scalar.dma_start(bk_f, bk.rearrange("(f p) -> p f", p=128))
            bv_st = constp.tile([1, 2 * FLOCc], F32, name="bv_st")
            nc.scalar.dma_start(bv_st[:, 0:FLOCc], bv.rearrange("(a b) -> a b", a=1))
            nc.scalar.dma_start(bv_st[:, FLOCc : 2 * FLOCc], bo.rearrange("(a b) -> a b", a=1))
            nc.vector.tensor_copy(bvb, bv_st[:, 0:FLOCc])
            nc.vector.tensor_copy(bob, bv_st[:, FLOCc : 2 * FLOCc])
            # ones columns of Vb
            nc.vector.memset(
                Vb.rearrange("p t (h u) -> p t h u", u=HDc + 1)[:, :, :, HDc : HDc + 1], 1.0
            )

        # ---------------- weight + x staging
        # All casts fp32->bf16 happen inside gpsimd (SWDGE) DMAs on the Pool
        # queue (serial per-DMA desc-gen; transfers serialize on DMA_ENGINES).
        # Order = first-use order: two 256-token x casts feed the transposes
        # gating the first qkv matmuls, then wqkv arrives per 128-row chunk so
        # the c-loop of the first Q/K groups is paced by chunk arrival instead
        # of one monolithic 3MB transfer.
        xv = x.rearrange("(b p2 p) dm -> b p p2 dm", p=128, p2=2)

        def stage_xpair(b2):
            xbf = sp.tile([128, 2, DMc], BF16, tag="xbf", name="xbf")
            nc.gpsimd.dma_start(xbf, xv[b2])
            for k in range(2):
                nc.sync.dma_start(
                    xT[:, :, ts(2 * b2 + k, 128)], xbf[:, k, :], transpose=True
                )

        wv_ = wqkv.rearrange("(c p) f -> c p f", p=128)
        emit_consts()
        stage_xpair(0)
        stage_xpair(1)
        for c in range(NDM):
            nc.gpsimd.dma_start(wqkvb[:, c, :], wv_[c])
        stage_xpair(2)
        stage_xpair(3)
        for b2 in range(4, NT // 2):
            stage_xpair(b2)
        nc.gpsimd.dma_start(wob, wo.rearrange("(c p) f -> p c f", p=128))

        # ---------------- qkv projection (one 512-token block)
        def copy_v(tt, psv):
            # NOTE: Pool/gpsimd has no PSUM access; PSUM reads must go
            # through Act or DVE.
            nc.scalar.copy(
                Vb[:, tt, :].rearrange("p (h u) -> p h u", u=HDc + 1)[:, :, 0:HDc],
                psv.rearrange("p (h d) -> p h d", d=HDc),
            )

        def qkv_block(tb, paced=False):
            qk = [psum.tile([128, 1024], F32, tag="stile", name=f"qk{ft}")
                  for ft in range(NFT)]
            # chunk-major emission: all four Q/K accumulation groups advance
            # together so PE work is available as soon as each w chunk lands.
            # In paced mode (first block, racing the weight-chunk DMAs), two
            # V accumulations join the chunk loop so per-chunk PE demand
            # matches the chunk arrival cadence.
            paced_tts = [tb * 4, tb * 4 + 1] if paced else []
            psvs = {tt: psum.tile([128, 512], F32, tag="work", name="psv_full")
                    for tt in paced_tts}
            for c in range(NDM):
                for ft in range(NFT):
                    nc.tensor.matmul(
                        qk[ft][:, 0:512], wqb[:, c, ts(ft, 128)], xT[:, c, ts(tb, 512)],
                        start=(c == 0), stop=(c == NDM - 1),
                    )
                    nc.tensor.matmul(
                        qk[ft][:, 512:1024], wkb[:, c, ts(ft, 128)], xT[:, c, ts(tb, 512)],
                        start=(c == 0), stop=(c == NDM - 1),
                    )
                for tt in paced_tts:
                    nc.tensor.matmul(
                        psvs[tt][:, 0:FLOCc], xT[:, c, ts(tt, 128)], wvb[:, c, :],
                        start=(c == 0), stop=False,
                    )
            for ft in range(NFT):
                nc.scalar.activation(QT[:, ft, ts(tb, 512)], qk[ft][:, 0:512],
                                     mybir.ActivationFunctionType.Identity,
                                     bias=bq_f[:, ft : ft + 1])
                nc.scalar.activation(KT[:, ft, ts(tb, 512)], qk[ft][:, 512:1024],
                                     mybir.ActivationFunctionType.Identity,
                                     bias=bk_f[:, ft : ft + 1])
            for tt in paced_tts:
                nc.tensor.matmul(psvs[tt][:, 0:FLOCc], ones_r, bvb, start=False, stop=True)
                copy_v(tt, psvs[tt][:, 0:FLOCc])
            for tt in range(tb * 4 + len(paced_tts), tb * 4 + 4):
                psv_full = psum.tile([128, 512], F32, tag="work", name="psv_full")
                psv = psv_full[:, 0:FLOCc]
                for c in range(NDM):
                    nc.tensor.matmul(
                        psv, xT[:, c, ts(tt, 128)], wvb[:, c, :],
                        start=(c == 0), stop=False,
                    )
                nc.tensor.matmul(psv, ones_r, bvb, start=False, stop=True)
                copy_v(tt, psv)

        # ---------------- attention helpers
        def emit_scores(p, h, i):
            hf, hp = h // 2, h % 2
            S = psum.tile([128, PW], F32, tag="stile", name="S")
            for j2 in range(NSUP):
                qs = p * PW + 512 * j2
                if 128 * i < qs + 512:
                    # causal: columns below the diagonal are never computed
                    al = max(0, 128 * i - qs)
                    nc.tensor.matmul(
                        S[:, ds(512 * j2 + al, 512 - al)],
                        KT[64 * hp : 64 * hp + 64, hf, ts(i, 128)],
                        QT[64 * hp : 64 * hp + 64, hf, ds(qs + al, 512 - al)],
                        start=True, stop=True,
                    )
            return S

        # AllGather + out-proj per 512-token super, split so the collective
        # can be emitted early and the PE-side projection late.
        def emit_ag(p, j2):
            q0 = p * PW + 512 * j2
            ag_in = dramp.tile([NFT * 128, 512], BF16, tag="agin", name="ag_in")
            # NOTE: Shared-output collectives need >4 cores/group; with
            # 4-core groups the output must be a Local scratch tensor.
            ag_out = dramp.tile([GRP * NFT * 128, 512], BF16, tag="agout", name="ag_out")
            for t in range(NFT):
                nc.sync.dma_start(ag_in[ts(t, 128), :], OTs[:, t, ds(q0, 512)])
            nc.gpsimd.collective_compute(
                "AllGather",
                mybir.AluOpType.bypass,
                ins=[ag_in.opt()],
                outs=[ag_out.opt()],
                replica_groups=groups,
            )
            return ag_out

        def emit_proj(p, j2, ag_out):
            q0 = p * PW + 512 * j2
            OF = ofp.tile([128, NDM, 512], BF16, tag="of", name="OF")
            # 2-chunk loads: SP-SEQ DMA issue is ~565ns/DMA, so per-chunk
            # loads would gate the projection on issue rate
            agv = ag_out.rearrange("(c p) q -> p c q", p=128)
            for c2 in range(NDM // 2):
                nc.sync.dma_start(OF[:, 2 * c2 : 2 * c2 + 2, :],
                                  agv[:, 2 * c2 : 2 * c2 + 2, :])
            osb = osbp.tile([128, 4, FLOCc], F32, tag="osb", name="osb")
            outv = out[ds(q0, 512), :].rearrange("(t p) f -> p t f", p=128)
            for ttl in range(4):
                pout_full = psum.tile([128, 512], F32, tag="work", name="pout_full")
                pout = pout_full[:, 0:FLOCc]
                for c in range(NDM):
                    nc.tensor.matmul(
                        pout, OF[:, c, ts(ttl, 128)], wob[:, c, :],
                        start=(c == 0), stop=False,
                    )
                nc.tensor.matmul(pout, ones_r, bob, start=False, stop=True)
                nc.vector.tensor_copy(osb[:, ttl, :], pout)
                if ttl == 1:
                    nc.sync.dma_start(outv[:, 0:2, :], osb[:, 0:2, :])
            nc.sync.dma_start(outv[:, 2:4, :], osb[:, 2:4, :])

        # ---------------- one attention pass (job stream over (head, k-tile))
        def attention_pass(p, inject=None, post=None):
            inject = inject or {}
            ilast = (p + 1) * PW // 128 - 1
            jobs = [(h, i) for h in range(HLOCc) for i in range(ilast + 1)]
            po_all = {}
            # one flat (h, i) stream with scores emitted one step ahead:
            # PE.SEQ is in-order, so S(next) must be issued before attnV(cur)
            # parks the queue on exp(cur) -- including across head boundaries.
            S_next = emit_scores(p, *jobs[0])
            for idx, (h, i) in enumerate(jobs):
                if idx in inject:
                    # fence so the scheduler keeps the AG/proj splice exactly
                    # where the emission order puts it
                    tc.no_sync_barrier()
                for fn in inject.get(idx, ()):
                    fn()
                hf, hp = h // 2, h % 2
                S = S_next
                if idx + 1 < len(jobs):
                    S_next = emit_scores(p, *jobs[idx + 1])
                if i == 0:
                    po_all[h] = [psum.tile([HDc + 1, 512], F32, tag="otile", name="po")
                                 for _ in range(NSUP)]
                po = po_all[h]
                astart = 128 * i - p * PW  # >=0 iff diagonal block in this pass
                es = max(0, astart)
                P = pbp.tile([128, PW], BF16, tag="ptile", name="P")
                nc.scalar.activation(
                    P[:, ds(es, PW - es)],
                    S[:, ds(es, PW - es)],
                    mybir.ActivationFunctionType.Exp,
                    scale=float(cfg.scale),
                )
                if astart >= 0:
                    nc.vector.tensor_mul(P[:, ds(astart, 128)], P[:, ds(astart, 128)], trimask)
                # attnV: trimmed to the causal range [al, 512) per super; the
                # masked sub-diagonal region of P is never read, so no
                # zero-fill of P is needed.  Diagonal super last so the
                # off-diagonal matmuls depend only on exp, not the mask-mul.
                j2s = [j2 for j2 in range(NSUP) if 128 * i < p * PW + 512 * j2 + 512]
                j2s = ([j2 for j2 in j2s if p * PW + 512 * j2 > 128 * i]
                       + [j2 for j2 in j2s if p * PW + 512 * j2 <= 128 * i])
                for j2 in j2s:
                    qs = p * PW + 512 * j2
                    al = max(0, 128 * i - qs)
                    ilastc = min(ilast, (qs + 512) // 128 - 1)
                    nc.tensor.matmul(
                        po[j2][:, ds(al, 512 - al)],
                        Vb[:, i, ds((HDc + 1) * h, HDc + 1)],
                        P[:, ds(512 * j2 + al, 512 - al)],
                        start=(i == 0), stop=(i == ilastc),
                    )
                    if i == ilastc:
                        # the copy exists to free the PSUM accumulator for the
                        # next head; the last head of the last pass normalizes
                        # straight from PSUM (shorter end-of-kernel chain)
                        if p == NPASS - 1 and h == HLOCc - 1:
                            osrc = po[j2]
                        else:
                            osrc = nrm.tile([HDc + 1, 512], F32, tag="osnap", name="osnap")
                            nc.vector.tensor_copy(osrc, po[j2])
                        rec = nrm.tile([1, 512], F32, tag="rec", name="rec")
                        nc.vector.reciprocal(rec, osrc[HDc : HDc + 1, :])
                        rb = nrm.tile([64, 512], F32, tag="rb", name="rb")
                        nc.gpsimd.partition_broadcast(rb, rec)
                        nc.vector.tensor_mul(
                            OTs[64 * hp : 64 * hp + 64, hf, ds(p * PW + 512 * j2, 512)],
                            osrc[0:HDc, :],
                            rb,
                        )
            if post:
                tc.no_sync_barrier()
            for fn in post or ():
                fn()

        # ---------------- phase schedule
        # qkv(tb0,tb1) -> pass0 (q < PW needs only the first PW of tokens)
        # -> qkv(tb2,tb3) (x casts for these blocks land during pass0)
        # -> pass1, with pass0's AG emitted early in the stream, its proj
        # mid-stream, and pass1's own supers drained as soon as each one's
        # last k-tile is done.
        blocks_per_pass = PW // 512
        for tb in range(blocks_per_pass):
            qkv_block(tb, paced=(tb == 0))
        aghold = {}

        def mk_ag(p, j2):
            def fn():
                aghold[(p, j2)] = emit_ag(p, j2)
            return fn

        def mk_proj(p, j2):
            def fn():
                emit_proj(p, j2, aghold.pop((p, j2)))
            return fn

        for p in range(NPASS):
            if p > 0:
                # fences: without them the Tile scheduler hoists these qkv
                # blocks before the previous attention pass, parking the
                # in-order PE queue on the late x transposes
                tc.no_sync_barrier()
                for tb in range(p * blocks_per_pass, min((p + 1) * blocks_per_pass, NB)):
                    qkv_block(tb)
                tc.no_sync_barrier()
            ilast = (p + 1) * PW // 128 - 1
            njobs = HLOCc * (ilast + 1)
            inject = {}

            def add(idx, fn):
                idx = max(0, min(idx, njobs - 1))
                inject.setdefault(idx, []).append(fn)

            if p > 0:
                # previous pass's AG right away (collective overlaps compute),
                # projection once the gather has certainly landed
                for j2 in range(NSUP):
                    add(2 + 2 * j2, mk_ag(p - 1, j2))
                add(11 * njobs // 32, mk_proj(p - 1, 0))
                if NSUP > 1:
                    add(21 * njobs // 32, mk_proj(p - 1, 1))
            post = []
            if p == NPASS - 1:
                # drain this pass's own supers: emit each AG as soon as its
                # last job is done (collective runs under the remaining jobs),
                # but keep ALL projections after the last AG emission -- a
                # projection spliced mid-stream parks the in-order PE queue
                # on its OF load while jobs could still run
                projs = []
                for j2 in range(NSUP):
                    qs = p * PW + 512 * j2
                    ilc = min(ilast, (qs + 512) // 128 - 1)
                    idx_done = (HLOCc - 1) * (ilast + 1) + ilc + 1
                    if idx_done < njobs:
                        add(idx_done, mk_ag(p, j2))
                    else:
                        post.append(mk_ag(p, j2))
                    projs.append(mk_proj(p, j2))
                post.extend(projs)
            attention_pass(p, inject=inject, post=post)
        psum_cm.__exit__(None, None, None)


def make_program(cfg=None, groups=None, unroll=1):
    cfg = cfg or Cfg()
    groups = groups or REPLICA_GROUPS
    nc = bacc.Bacc("TRN2", target_bir_lowering=False, debug=False, num_devices=NCORES)
    x = nc.dram_tensor("x", [cfg.L, cfg.DM], F32, kind="ExternalInput").ap()
    wqkv = nc.dram_tensor("wqkv", [cfg.DM, 3 * cfg.FLOC], F32, kind="ExternalInput").ap()
    bq = nc.dram_tensor("bq", [cfg.FLOC], F32, kind="ExternalInput").ap()
    bk = nc.dram_tensor("bk", [cfg.FLOC], F32, kind="ExternalInput").ap()
    bv = nc.dram_tensor("bv", [cfg.FLOC], F32, kind="ExternalInput").ap()
    wo = nc.dram_tensor("wo", [cfg.DM, cfg.FLOC], F32, kind="ExternalInput").ap()
    bo = nc.dram_tensor("bo", [cfg.FLOC], F32, kind="ExternalInput").ap()
    out = nc.dram_tensor("out", [cfg.L, cfg.FLOC], F32, kind="ExternalOutput").ap()
    with tile.TileContext(nc) as tc:
        nc.tc = tc
        for _ in range(unroll):
            build_body(nc, cfg, x, wqkv, bq, bk, bv, wo, bo, out, groups)
    nc.compile()
    return nc


def shard_inputs(x, w_qkv, b_qkv, w_out, b_out, cfg=None):
    """Full inputs -> list of 8 per-core input dicts."""
    cfg = cfg or Cfg()
    FL = cfg.FLOC
    DMF = cfg.DM
    in_maps = []
    for c in range(NCORES):
        b, r = divmod(c, GRP)
        q0 = r * FL
        in_maps.append({
            "x": np.ascontiguousarray(x[b]),
            "wqkv": np.ascontiguousarray(np.concatenate([
                w_qkv[:, q0 : q0 + FL],
                w_qkv[:, DMF + q0 : DMF + q0 + FL],
                w_qkv[:, 2 * DMF + q0 : 2 * DMF + q0 + FL],
            ], axis=1)),
            "bq": np.ascontiguousarray(b_qkv[q0 : q0 + FL]),
            "bk": np.ascontiguousarray(b_qkv[DMF + q0 : DMF + q0 + FL]),
            "bv": np.ascontiguousarray(b_qkv[2 * DMF + q0 : 2 * DMF + q0 + FL]),
            "wo": np.ascontiguousarray(w_out[:, q0 : q0 + FL]),
            "bo": np.ascontiguousarray(b_out[q0 : q0 + FL]),
        })
    return in_maps


def gather_output(results, cfg=None):
    cfg = cfg or Cfg()
    FL = cfg.FLOC
    out = np.empty((BS, cfg.L, cfg.DM), np.float32)
    for c in range(NCORES):
        b, r = divmod(c, GRP)
        out[b, :, r * FL : (r + 1) * FL] = results[c]["out"]
    return out


_PROGRAM = None


def _get_program():
    global _PROGRAM
    if _PROGRAM is None:
        _PROGRAM = make_program()
    return _PROGRAM


def kernel(x, w_qkv, b_qkv, w_out, b_out):
    x = np.asarray(x, np.float32)
    w_qkv = np.asarray(w_qkv, np.float32)
    b_qkv = np.asarray(b_qkv, np.float32)
    w_out = np.asarray(w_out, np.float32)
    b_out = np.asarray(b_out, np.float32)
    nc = _get_program()
    in_maps = shard_inputs(x, w_qkv, b_qkv, w_out, b_out)
    res = run_bass_kernel_spmd(nc, in_maps, list(range(NCORES)))
    return gather_output(res.results)
